# revision 46
# baseline (speedup 1.0000x reference)
"""Trainium2 Bass kernel for nn_CombinatorialClassifier.

Computation (reference):
    logits = einsum('bf,pqf->bpq', x, W) + b        # [B,P,Q]
    logp   = log_softmax(logits, axis=2)            # [B,P,Q]
    out    = take_along_axis(logp, part_idx, 2)     # [B,P,C]

Shapes: B=256, P=64, Q=128, C=1000, F=2048.

Sharding: expert-parallel over P across 8 cores (8 partitionings per
core).  Each core reads the full x and its W/b/part_idx slice and
writes its disjoint [B, 8, C] slice of the output.  No collectives.

Per-core dataflow (PSUM orientation [q, b] for the linear part):
  - W arrives per-p (one DMA per partitioning, x combined with W0) so
    p=0's matmuls start after ~1.7MB of DMA instead of the full 7MB.
  - lin group: bias K=1 matmul opens, 16 k-tile matmuls accumulate;
    after exp/sumexp/ln a final K=1 matmul (negones[q] x lse[b]) adds
    -lse[b] to every element, so psum_lin holds log-softmax directly.
  - gather: psum_out[b, c] = logpT[q,b].T @ OH[q,c] with the one-hot
    OH built on the HOST (exact 0/1) and shipped as an input; the
    PSUM->SBUF drains are then PLAIN dtype-cast copies (single dep),
    alternating DVE / ACT per p-pair.
  - output staged in SBUF fp16 (halves out traffic); host casts back
    to fp32.

This walrus build fits only ONE sync-wait per instruction.  Instead of
contorting the dataflow, _install_wait_split post-processes the
serialized IR: every instruction with N>1 waits keeps one and gets
N-1 standalone single-wait EventSemaphore instructions immediately
before it on the same engine/queue — semantically identical.
"""

import numpy as np

B, P, Q, C, F = 256, 64, 128, 1000, 2048
NCORES = 8
PL = P // NCORES          # partitionings per core
KT = F // 128             # contraction tiles
BT = B // 128             # batch tiles for the gather matmul
C_CHUNKS = [(0, 512), (512, C - 512)]
KW = B + Q                # combined x|w0 column block per k-tile

# dtypes (mybir names) for the big streamed operands
X_DT = "float8e4"         # x (moving in the main matmul; DoubleRow)
W_DT = "float8e4"         # W (stationary in main matmul; DoubleRow)
OH_DT = "float8e4"        # one-hot gather matrix (moving; 0/1 exact)
GATHER_DP = True          # DoublePixel perf mode on the gather matmuls

# input DMA grouping: (name, list of p's) for W and OH group tiles,
# interleaved so each operand lands just before the PE needs it
W_GROUPS = [("w0", [0]), ("wa", [1, 2]), ("wb", [3, 4, 5]), ("wc", [6, 7])]
OH_GROUPS = [("oha", [0, 1]), ("ohb", [2, 3, 4]), ("ohc", [5, 6, 7])]


def _np_dt(name):
    import ml_dtypes
    return {
        "float16": np.float16,
        "bfloat16": ml_dtypes.bfloat16,
        "float8e4": ml_dtypes.float8_e4m3fn,
        "float32": np.float32,
    }[name]


def _build_nc():
    import concourse.bass as bass
    import concourse.tile as tile
    from concourse import mybir
    from contextlib import ExitStack

    DT = mybir.dt.float32
    HT = mybir.dt.float16
    XDT = getattr(mybir.dt, X_DT)
    WDT = getattr(mybir.dt, W_DT)
    OHDT = getattr(mybir.dt, OH_DT)
    AF = mybir.ActivationFunctionType

    nc = bass.Bass()
    const_d = nc.declare_dram_parameter(
        "const", [1, PL * Q + B + 128], HT, isOutput=False)
    x_d = nc.declare_dram_parameter("xin", [2, 128, (KT // 2) * B], XDT,
                                    isOutput=False)
    w_d = nc.declare_dram_parameter("win", [128, PL * KT * Q], WDT,
                                    isOutput=False)
    oh_d = nc.declare_dram_parameter("ohin", [128, PL * C], OHDT,
                                     isOutput=False)
    out_d = nc.declare_dram_parameter("out", [B, PL, C], HT, isOutput=True)

    OFF_BIAS = 0
    OFF_ONES = PL * Q
    OFF_NEG = PL * Q + B

    with ExitStack() as ctx:
        tc = ctx.enter_context(tile.TileContext(nc))
        singles = ctx.enter_context(tc.tile_pool(name="singles", bufs=1))
        ps_lin = ctx.enter_context(
            tc.tile_pool(name="ps_lin", bufs=3, space=bass.MemorySpace.PSUM))
        ps_sum = ctx.enter_context(
            tc.tile_pool(name="ps_sum", bufs=1, space=bass.MemorySpace.PSUM))
        ps_out = ctx.enter_context(
            tc.tile_pool(name="ps_out", bufs=2, space=bass.MemorySpace.PSUM))

        def fresh(shape, dtype, tag):
            return singles.tile(shape, dtype, tag=tag, name=tag)

        # ---- input DMAs (SP queue, in tuned arrival order) ----------
        x_a = fresh([128, KT // 2, B], XDT, "x_a")
        x_b = fresh([128, KT // 2, B], XDT, "x_b")
        const_sb = fresh([1, PL * Q + B + 128], HT, "const")
        w_tiles = {}
        oh_tiles = {}

        def dma_w_group(gi):
            name, ps = W_GROUPS[gi]
            t = fresh([128, len(ps) * KT, Q], WDT, name)
            nc.sync.dma_start(
                out=t[:],
                in_=w_d[:, ps[0] * KT * Q:(ps[-1] + 1) * KT * Q])
            for j, p in enumerate(ps):
                w_tiles[p] = (t, j)

        def dma_oh_group(gi):
            name, ps = OH_GROUPS[gi]
            t = fresh([128, len(ps), C], OHDT, name)
            nc.sync.dma_start(
                out=t[:], in_=oh_d[:, ps[0] * C:(ps[-1] + 1) * C])
            for j, p in enumerate(ps):
                oh_tiles[p] = (t, j)

        nc.sync.dma_start(out=const_sb[:], in_=const_d[:])
        nc.sync.dma_start(out=x_a[:], in_=x_d[0])
        dma_w_group(0)                     # w0
        nc.sync.dma_start(out=x_b[:], in_=x_d[1])
        dma_w_group(1)                     # w1-2
        dma_oh_group(0)                    # oh0-1
        dma_w_group(2)                     # w3-5
        dma_oh_group(1)                    # oh2-4
        dma_w_group(3)                     # w6-7
        dma_oh_group(2)                    # oh5-7

        def w_pair_slice(p, t2):
            t, j = w_tiles[p]
            return t[:, j * KT + 2 * t2:j * KT + 2 * t2 + 2, :]

        def x_pair_slice(t2):
            t = x_a if t2 < KT // 4 else x_b
            tt = t2 % (KT // 4)
            return t[:, 2 * tt:2 * tt + 2, :]

        def oh_slice(p, c0, cw):
            t, j = oh_tiles[p]
            return t[:, j, c0:c0 + cw]

        # ones column for the sumexp matmuls (ACT-made, dep on x_a DMA)
        ones_col = fresh([128, 1], HT, "ones_col")
        nc.scalar.activation(out=ones_col[:], in_=x_a[:, 0, 0:1],
                             func=AF.Copy, bias=1.0, scale=0.0)

        # ---- PE warm-up -------------------------------------------
        # The first real matmuls can't start until x/W stream in
        # (~3.5us after the queue opens), and TRN2's PE needs ~3us of
        # continuous work to DVFS from 1.2GHz to 2.4GHz.  Fill the dead
        # zone with throwaway K=1 matmuls that only need the (tiny,
        # first-in-queue) const DMA, so the real mains start at full
        # clock.  They write the ps_sum bank, which is otherwise unused
        # until p0's sumexp.
        # (standalone weight-loads touch no PSUM; they are emitted first
        # so the scheduler packs them into the dead zone, and every real
        # matmul reloads its own stationary afterwards)
        for _ in range(30):
            nc.tensor.ldweights(const_sb[0:1, 0:128])

        # ---- per-partitioning pipeline ------------------------------
        og_tiles = {}
        # drain-engine per og tile (pair, bt): DVE ~11 drains, ACT ~5,
        # interleaved in time so neither engine falls behind
        ACT_TILES = {(1, 0), (1, 1), (3, 0), (3, 1)}
        for p in range(PL):
            pair = p // 2

            psum_lin = ps_lin.tile([128, B], DT)
            nc.tensor.matmul(
                psum_lin[:],
                const_sb[:, OFF_BIAS + p * Q:OFF_BIAS + (p + 1) * Q],
                const_sb[:, OFF_ONES:OFF_ONES + B],
                start=True, stop=False)
            for t2 in range(KT // 2):
                # DoubleRow: two 128-deep k-tiles contract per matmul
                nc.tensor.matmul(
                    psum_lin[:], w_pair_slice(p, t2), x_pair_slice(t2),
                    start=False, stop=(t2 == KT // 2 - 1),
                    perf_mode=mybir.MatmulPerfMode.DoubleRow)

            expT = fresh([128, B], HT, f"exp{p}")
            nc.scalar.activation(out=expT[:], in_=psum_lin[:], func=AF.Exp)

            psum_sum = ps_sum.tile([1, B], DT)
            nc.tensor.matmul(
                psum_sum[:], ones_col[:], expT[:],
                start=True, stop=True)
            lse = fresh([1, B], HT, f"lse{p}")
            nc.scalar.activation(out=lse[:], in_=psum_sum[:], func=AF.Ln)

            # -lse folded into the linear psum: psum[q,b] += (-1)*lse[b]
            nc.tensor.matmul(
                psum_lin[:],
                const_sb[:, OFF_NEG:OFF_NEG + 128],
                lse[:],
                start=False, stop=True, skip_group_check=True)

            linT = fresh([128, B], HT, f"lin{p}")
            nc.vector.tensor_copy(linT[:], psum_lin[:])

            for bt in range(BT):
                bsl = slice(bt * 128, (bt + 1) * 128)
                if p % 2 == 0:
                    og_tiles[(pair, bt)] = fresh([128, 2, C], HT,
                                                 f"og{pair}_{bt}")
                og = og_tiles[(pair, bt)]
                psum_out = ps_out.tile([128, 1024], DT)
                for (c0, cw) in C_CHUNKS:
                    nc.tensor.matmul(
                        psum_out[:, c0:c0 + cw],
                        linT[:, bsl],
                        oh_slice(p, c0, cw),
                        start=True, stop=True,
                        perf_mode=(mybir.MatmulPerfMode.DoublePixel
                                   if GATHER_DP else None))
                if (pair, bt) not in ACT_TILES:
                    nc.vector.tensor_copy(og[:, p % 2, :], psum_out[:, :C])
                else:
                    nc.scalar.activation(out=og[:, p % 2, :],
                                         in_=psum_out[:, :C], func=AF.Copy)
                if p % 2 == 1:
                    # out-DMAs dispatched from the otherwise-idle GpSimd
                    # sequencer (own HWDGE queue, overlaps the input queue)
                    nc.gpsimd.dma_start(
                        out=out_d[bsl, p - 1:p + 1, :],
                        in_=og[:])

    _install_wait_split(nc)
    return nc


def _install_wait_split(nc):
    """Walrus fits ONE sync-wait per instruction.  For every instruction
    carrying N>1 waits, keep the last and emit N-1 standalone
    EventSemaphore instructions (same engine, one wait each) before it.
    Engines execute their stream in order, so this is semantically
    identical.  Applied at serialization time so every consumer of
    nc.to_json_bytes() sees the legal form."""
    import json

    orig = nc.to_json_bytes

    def patched():
        m = json.loads(orig())
        n_split = 0
        for fn in m["functions"]:
            for bb in fn["blocks"]:
                out = []
                for inst in bb["instructions"]:
                    si = inst.get("sync_info")
                    if si and si.get("on_wait") and len(si["on_wait"]) > 1:
                        waits = si["on_wait"]
                        head, keep = waits[:-1], waits[-1:]
                        for j, w in enumerate(head):
                            out.append({
                                "debug": inst.get("debug", 0),
                                "engine": inst["engine"],
                                "ins": [],
                                "name": f"{inst['name']}-ws{j}",
                                "opcode": "EventSemaphore",
                                "outs": [],
                                "sync_info": {
                                    "on_update": [],
                                    "on_wait": [w],
                                },
                            })
                            n_split += 1
                        si["on_wait"] = keep
                    out.append(inst)
                bb["instructions"] = out
        return json.dumps(m).encode()

    nc.to_json_bytes = patched


def _host_inputs(x, W, b, part_idx):
    """Build the 8 per-core input maps."""
    x_np = _np_dt(X_DT)
    w_np = _np_dt(W_DT)
    oh_np = _np_dt(OH_DT)

    # x: [2, 128 f_in, (KT/2)*B]
    xh = np.ascontiguousarray(
        x.T.reshape(2, KT // 2, 128, B).transpose(0, 2, 1, 3)
         .reshape(2, 128, (KT // 2) * B)).astype(x_np)
    qs = np.arange(Q)
    in_maps = []
    for i in range(NCORES):
        sl = slice(i * PL, (i + 1) * PL)
        Wc = W[sl]                                     # [PL, Q, F]
        # -> [128 f_in, PL, KT, Q] -> [128, PL*KT*Q]
        wh = np.ascontiguousarray(
            Wc.transpose(2, 0, 1).reshape(KT, 128, PL, Q)
              .transpose(1, 2, 0, 3).reshape(128, PL * KT * Q)).astype(w_np)
        idx = part_idx[sl]                             # [PL, C]
        # oh[q, p, c] -> [128, PL*C]
        oh = np.ascontiguousarray(
            (idx[None, :, :] == qs[:, None, None])
            .reshape(128, PL * C)).astype(oh_np)
        const = np.zeros((1, PL * Q + B + 128), dtype=np.float16)
        const[0, :PL * Q] = b[sl].reshape(-1).astype(np.float16)
        const[0, PL * Q:PL * Q + B] = 1.0
        const[0, PL * Q + B:] = -1.0
        in_maps.append({
            "const": const,
            "xin": xh,
            "win": wh,
            "ohin": oh,
        })
    return in_maps


def kernel(x, W, b, part_idx, _trace=False):
    from concourse.bass_utils import run_bass_kernel_spmd

    x = np.asarray(x, dtype=np.float32)
    W = np.asarray(W, dtype=np.float32)
    b = np.asarray(b, dtype=np.float32)
    part_idx = np.asarray(part_idx)

    nc = _build_nc()
    in_maps = _host_inputs(x, W, b, part_idx)
    res = run_bass_kernel_spmd(nc, in_maps, list(range(NCORES)),
                               trace=_trace)
    out = np.concatenate(
        [r["out"].astype(np.float32) for r in res.results], axis=1)
    if _trace:
        return out, res
    return out


# revision 54
# speedup vs baseline: 1.0080x; 1.0080x over previous
"""Trainium2 Bass kernel for nn_CombinatorialClassifier.

Computation (reference):
    logits = einsum('bf,pqf->bpq', x, W) + b        # [B,P,Q]
    logp   = log_softmax(logits, axis=2)            # [B,P,Q]
    out    = take_along_axis(logp, part_idx, 2)     # [B,P,C]

Shapes: B=256, P=64, Q=128, C=1000, F=2048.

Sharding: expert-parallel over P across 8 cores (8 partitionings per
core).  Each core reads the full x and its W/b/part_idx slice and
writes its disjoint [B, 8, C] slice of the output.  No collectives.

Per-core dataflow (PSUM orientation [q, b] for the linear part):
  - W arrives per-p (one DMA per partitioning, x combined with W0) so
    p=0's matmuls start after ~1.7MB of DMA instead of the full 7MB.
  - lin group: bias K=1 matmul opens, 16 k-tile matmuls accumulate;
    after exp/sumexp/ln a final K=1 matmul (negones[q] x lse[b]) adds
    -lse[b] to every element, so psum_lin holds log-softmax directly.
  - gather: psum_out[b, c] = logpT[q,b].T @ OH[q,c] with the one-hot
    OH built on the HOST (exact 0/1) and shipped as an input; the
    PSUM->SBUF drains are then PLAIN dtype-cast copies (single dep),
    alternating DVE / ACT per p-pair.
  - output staged in SBUF fp16 (halves out traffic); host casts back
    to fp32.

This walrus build fits only ONE sync-wait per instruction.  Instead of
contorting the dataflow, _install_wait_split post-processes the
serialized IR: every instruction with N>1 waits keeps one and gets
N-1 standalone single-wait EventSemaphore instructions immediately
before it on the same engine/queue — semantically identical.
"""

import numpy as np

B, P, Q, C, F = 256, 64, 128, 1000, 2048
NCORES = 8
PL = P // NCORES          # partitionings per core
KT = F // 128             # contraction tiles
BT = B // 128             # batch tiles for the gather matmul
C_CHUNKS = [(0, 512), (512, C - 512)]
KW = B + Q                # combined x|w0 column block per k-tile

# dtypes (mybir names) for the big streamed operands
X_DT = "float8e4"         # x (moving in the main matmul; DoubleRow)
W_DT = "float8e4"         # W (stationary in main matmul; DoubleRow)
OH_DT = "float8e4"        # one-hot gather matrix (moving; 0/1 exact)
GATHER_DP = True          # DoublePixel perf mode on the gather matmuls

# input DMA grouping: (name, list of p's) for W and OH group tiles,
# interleaved so each operand lands just before the PE needs it
W_GROUPS = [("w0", [0]), ("wa", [1, 2]), ("wb", [3, 4, 5]), ("wc", [6, 7])]
OH_GROUPS = [("oha", [0, 1]), ("ohb", [2, 3, 4]), ("ohc", [5, 6, 7])]


def _np_dt(name):
    import ml_dtypes
    return {
        "float16": np.float16,
        "bfloat16": ml_dtypes.bfloat16,
        "float8e4": ml_dtypes.float8_e4m3fn,
        "float32": np.float32,
    }[name]


def _build_nc():
    import concourse.bass as bass
    import concourse.tile as tile
    from concourse import mybir
    from contextlib import ExitStack

    DT = mybir.dt.float32
    HT = mybir.dt.float16
    XDT = getattr(mybir.dt, X_DT)
    WDT = getattr(mybir.dt, W_DT)
    OHDT = getattr(mybir.dt, OH_DT)
    AF = mybir.ActivationFunctionType

    nc = bass.Bass()
    const_d = nc.declare_dram_parameter(
        "const", [1, PL * Q + B + 128], HT, isOutput=False)
    x_d = nc.declare_dram_parameter("xin", [2, 128, (KT // 2) * B], XDT,
                                    isOutput=False)
    w_d = nc.declare_dram_parameter("win", [128, PL * KT * Q], WDT,
                                    isOutput=False)
    oh_d = nc.declare_dram_parameter("ohin", [128, PL * C], OHDT,
                                     isOutput=False)
    out_d = nc.declare_dram_parameter("out", [B, PL, C], HT, isOutput=True)

    OFF_BIAS = 0
    OFF_ONES = PL * Q
    OFF_NEG = PL * Q + B

    with ExitStack() as ctx:
        tc = ctx.enter_context(tile.TileContext(nc))
        singles = ctx.enter_context(tc.tile_pool(name="singles", bufs=1))
        ps_lin = ctx.enter_context(
            tc.tile_pool(name="ps_lin", bufs=3, space=bass.MemorySpace.PSUM))
        ps_sum = ctx.enter_context(
            tc.tile_pool(name="ps_sum", bufs=1, space=bass.MemorySpace.PSUM))
        ps_out = ctx.enter_context(
            tc.tile_pool(name="ps_out", bufs=2, space=bass.MemorySpace.PSUM))

        def fresh(shape, dtype, tag):
            return singles.tile(shape, dtype, tag=tag, name=tag)

        # ---- input DMAs (SP queue, in tuned arrival order) ----------
        x_a = fresh([128, KT // 2, B], XDT, "x_a")
        x_b = fresh([128, KT // 2, B], XDT, "x_b")
        const_sb = fresh([1, PL * Q + B + 128], HT, "const")
        w_tiles = {}
        oh_tiles = {}

        def dma_w_group(gi):
            name, ps = W_GROUPS[gi]
            t = fresh([128, len(ps) * KT, Q], WDT, name)
            nc.sync.dma_start(
                out=t[:],
                in_=w_d[:, ps[0] * KT * Q:(ps[-1] + 1) * KT * Q])
            for j, p in enumerate(ps):
                w_tiles[p] = (t, j)

        def dma_oh_group(gi):
            name, ps = OH_GROUPS[gi]
            t = fresh([128, len(ps), C], OHDT, name)
            nc.sync.dma_start(
                out=t[:], in_=oh_d[:, ps[0] * C:(ps[-1] + 1) * C])
            for j, p in enumerate(ps):
                oh_tiles[p] = (t, j)

        nc.sync.dma_start(out=const_sb[:], in_=const_d[:])
        nc.sync.dma_start(out=x_a[:], in_=x_d[0])
        dma_w_group(0)                     # w0
        nc.sync.dma_start(out=x_b[:], in_=x_d[1])
        dma_w_group(1)                     # w1-2
        dma_oh_group(0)                    # oh0-1
        dma_w_group(2)                     # w3-5
        dma_oh_group(1)                    # oh2-4
        dma_w_group(3)                     # w6-7
        dma_oh_group(2)                    # oh5-7

        def w_pair_slice(p, t2):
            t, j = w_tiles[p]
            return t[:, j * KT + 2 * t2:j * KT + 2 * t2 + 2, :]

        def x_pair_slice(t2):
            t = x_a if t2 < KT // 4 else x_b
            tt = t2 % (KT // 4)
            return t[:, 2 * tt:2 * tt + 2, :]

        def oh_slice(p, c0, cw):
            t, j = oh_tiles[p]
            return t[:, j, c0:c0 + cw]

        # ones column for the sumexp matmuls (ACT-made, dep on x_a DMA)
        ones_col = fresh([128, 1], HT, "ones_col")
        nc.scalar.activation(out=ones_col[:], in_=x_a[:, 0, 0:1],
                             func=AF.Copy, bias=1.0, scale=0.0)

        # ---- per-partitioning pipeline ------------------------------
        og_tiles = {}
        # drain-engine per og tile (pair, bt): DVE ~11 drains, ACT ~5,
        # interleaved in time so neither engine falls behind
        ACT_TILES = {(1, 0), (1, 1), (3, 0), (3, 1)}
        last_main = {}
        gather_after = {}
        for p in range(PL):
            pair = p // 2

            psum_lin = ps_lin.tile([128, B], DT)
            nc.tensor.matmul(
                psum_lin[:],
                const_sb[:, OFF_BIAS + p * Q:OFF_BIAS + (p + 1) * Q],
                const_sb[:, OFF_ONES:OFF_ONES + B],
                start=True, stop=False)
            for t2 in range(KT // 2):
                # DoubleRow: two 128-deep k-tiles contract per matmul
                mmi = nc.tensor.matmul(
                    psum_lin[:], w_pair_slice(p, t2), x_pair_slice(t2),
                    start=False, stop=(t2 == KT // 2 - 1),
                    perf_mode=mybir.MatmulPerfMode.DoubleRow)
            last_main[p] = mmi
            for g in gather_after.pop(p - 1, []):
                tile.add_dep_helper(g.ins, mmi.ins, sync=False,
                                    reason="gather(p) after mains(p+1)")

            expT = fresh([128, B], HT, f"exp{p}")
            nc.scalar.activation(out=expT[:], in_=psum_lin[:], func=AF.Exp)

            psum_sum = ps_sum.tile([1, B], DT)
            nc.tensor.matmul(
                psum_sum[:], ones_col[:], expT[:],
                start=True, stop=True)
            lse = fresh([1, B], HT, f"lse{p}")
            nc.scalar.activation(out=lse[:], in_=psum_sum[:], func=AF.Ln)

            # -lse folded into the linear psum: psum[q,b] += (-1)*lse[b]
            nc.tensor.matmul(
                psum_lin[:],
                const_sb[:, OFF_NEG:OFF_NEG + 128],
                lse[:],
                start=False, stop=True, skip_group_check=True)

            linT = fresh([128, B], HT, f"lin{p}")
            nc.vector.tensor_copy(linT[:], psum_lin[:])

            for bt in range(BT):
                bsl = slice(bt * 128, (bt + 1) * 128)
                if p % 2 == 0:
                    og_tiles[(pair, bt)] = fresh([128, 2, C], HT,
                                                 f"og{pair}_{bt}")
                og = og_tiles[(pair, bt)]
                psum_out = ps_out.tile([128, 1024], DT)
                for (c0, cw) in C_CHUNKS:
                    gmi = nc.tensor.matmul(
                        psum_out[:, c0:c0 + cw],
                        linT[:, bsl],
                        oh_slice(p, c0, cw),
                        start=True, stop=True,
                        perf_mode=(mybir.MatmulPerfMode.DoublePixel
                                   if GATHER_DP else None))
                    if p + 1 < PL:
                        # order-only: keep gather(p) after mains(p+1) so
                        # premature gathers never stall the PE mid-ramp
                        gather_after.setdefault(p, []).append(gmi)
                if (pair, bt) not in ACT_TILES:
                    nc.vector.tensor_copy(og[:, p % 2, :], psum_out[:, :C])
                else:
                    nc.scalar.activation(out=og[:, p % 2, :],
                                         in_=psum_out[:, :C], func=AF.Copy)
                if p % 2 == 1:
                    # out-DMAs dispatched from the otherwise-idle GpSimd
                    # sequencer (own HWDGE queue, overlaps the input queue)
                    nc.gpsimd.dma_start(
                        out=out_d[bsl, p - 1:p + 1, :],
                        in_=og[:])

    _install_wait_split(nc)
    return nc


def _install_wait_split(nc):
    """Walrus fits ONE sync-wait per instruction.  For every instruction
    carrying N>1 waits, keep the last and emit N-1 standalone
    EventSemaphore instructions (same engine, one wait each) before it.
    Engines execute their stream in order, so this is semantically
    identical.  Applied at serialization time so every consumer of
    nc.to_json_bytes() sees the legal form."""
    import json

    orig = nc.to_json_bytes

    def patched():
        m = json.loads(orig())
        n_split = 0
        for fn in m["functions"]:
            for bb in fn["blocks"]:
                out = []
                for inst in bb["instructions"]:
                    si = inst.get("sync_info")
                    if si and si.get("on_wait") and len(si["on_wait"]) > 1:
                        waits = si["on_wait"]
                        head, keep = waits[:-1], waits[-1:]
                        for j, w in enumerate(head):
                            out.append({
                                "debug": inst.get("debug", 0),
                                "engine": inst["engine"],
                                "ins": [],
                                "name": f"{inst['name']}-ws{j}",
                                "opcode": "EventSemaphore",
                                "outs": [],
                                "sync_info": {
                                    "on_update": [],
                                    "on_wait": [w],
                                },
                            })
                            n_split += 1
                        si["on_wait"] = keep
                    out.append(inst)
                bb["instructions"] = out
        return json.dumps(m).encode()

    nc.to_json_bytes = patched


def _host_inputs(x, W, b, part_idx):
    """Build the 8 per-core input maps."""
    x_np = _np_dt(X_DT)
    w_np = _np_dt(W_DT)
    oh_np = _np_dt(OH_DT)

    # x: [2, 128 f_in, (KT/2)*B]
    xh = np.ascontiguousarray(
        x.T.reshape(2, KT // 2, 128, B).transpose(0, 2, 1, 3)
         .reshape(2, 128, (KT // 2) * B)).astype(x_np)
    qs = np.arange(Q)
    in_maps = []
    for i in range(NCORES):
        sl = slice(i * PL, (i + 1) * PL)
        Wc = W[sl]                                     # [PL, Q, F]
        # -> [128 f_in, PL, KT, Q] -> [128, PL*KT*Q]
        wh = np.ascontiguousarray(
            Wc.transpose(2, 0, 1).reshape(KT, 128, PL, Q)
              .transpose(1, 2, 0, 3).reshape(128, PL * KT * Q)).astype(w_np)
        idx = part_idx[sl]                             # [PL, C]
        # oh[q, p, c] -> [128, PL*C]
        oh = np.ascontiguousarray(
            (idx[None, :, :] == qs[:, None, None])
            .reshape(128, PL * C)).astype(oh_np)
        const = np.zeros((1, PL * Q + B + 128), dtype=np.float16)
        const[0, :PL * Q] = b[sl].reshape(-1).astype(np.float16)
        const[0, PL * Q:PL * Q + B] = 1.0
        const[0, PL * Q + B:] = -1.0
        in_maps.append({
            "const": const,
            "xin": xh,
            "win": wh,
            "ohin": oh,
        })
    return in_maps


def kernel(x, W, b, part_idx, _trace=False):
    from concourse.bass_utils import run_bass_kernel_spmd

    x = np.asarray(x, dtype=np.float32)
    W = np.asarray(W, dtype=np.float32)
    b = np.asarray(b, dtype=np.float32)
    part_idx = np.asarray(part_idx)

    nc = _build_nc()
    in_maps = _host_inputs(x, W, b, part_idx)
    res = run_bass_kernel_spmd(nc, in_maps, list(range(NCORES)),
                               trace=_trace)
    out = np.concatenate(
        [r["out"].astype(np.float32) for r in res.results], axis=1)
    if _trace:
        return out, res
    return out


# revision 60
# speedup vs baseline: 1.0275x; 1.0193x over previous
"""Trainium2 Bass kernel for nn_CombinatorialClassifier.

Computation (reference):
    logits = einsum('bf,pqf->bpq', x, W) + b        # [B,P,Q]
    logp   = log_softmax(logits, axis=2)            # [B,P,Q]
    out    = take_along_axis(logp, part_idx, 2)     # [B,P,C]

Shapes: B=256, P=64, Q=128, C=1000, F=2048.

Sharding: expert-parallel over P across 8 cores (8 partitionings per
core).  Each core reads the full x and its W/b/part_idx slice and
writes its disjoint [B, 8, C] slice of the output.  No collectives.

Per-core dataflow (PSUM orientation [q, b] for the linear part):
  - W arrives per-p (one DMA per partitioning, x combined with W0) so
    p=0's matmuls start after ~1.7MB of DMA instead of the full 7MB.
  - lin group: bias K=1 matmul opens, 16 k-tile matmuls accumulate;
    after exp/sumexp/ln a final K=1 matmul (negones[q] x lse[b]) adds
    -lse[b] to every element, so psum_lin holds log-softmax directly.
  - gather: psum_out[b, c] = logpT[q,b].T @ OH[q,c] with the one-hot
    OH built on the HOST (exact 0/1) and shipped as an input; the
    PSUM->SBUF drains are then PLAIN dtype-cast copies (single dep),
    alternating DVE / ACT per p-pair.
  - output staged in SBUF fp16 (halves out traffic); host casts back
    to fp32.

This walrus build fits only ONE sync-wait per instruction.  Instead of
contorting the dataflow, _install_wait_split post-processes the
serialized IR: every instruction with N>1 waits keeps one and gets
N-1 standalone single-wait EventSemaphore instructions immediately
before it on the same engine/queue — semantically identical.
"""

import numpy as np

B, P, Q, C, F = 256, 64, 128, 1000, 2048
NCORES = 8
PL = P // NCORES          # partitionings per core
KT = F // 128             # contraction tiles
BT = B // 128             # batch tiles for the gather matmul
C_CHUNKS = [(0, 512), (512, C - 512)]
KW = B + Q                # combined x|w0 column block per k-tile

# dtypes (mybir names) for the big streamed operands
X_DT = "float8e4"         # x (moving in the main matmul; DoubleRow)
W_DT = "float8e4"         # W (stationary in main matmul; DoubleRow)
OH_DT = "float8e4"        # one-hot gather matrix (moving; 0/1 exact)
GATHER_DP = True          # DoublePixel perf mode on the gather matmuls

# input DMA grouping: (name, list of p's) for W and OH group tiles,
# interleaved so each operand lands just before the PE needs it
W_GROUPS = [("w0", [0]), ("wa", [1, 2]), ("wb", [3, 4, 5]), ("wc", [6, 7])]
OH_GROUPS = [("oha", [0, 1]), ("ohb", [2, 3, 4]), ("ohc", [5, 6, 7])]


def _np_dt(name):
    import ml_dtypes
    return {
        "float16": np.float16,
        "bfloat16": ml_dtypes.bfloat16,
        "float8e4": ml_dtypes.float8_e4m3fn,
        "float32": np.float32,
    }[name]


def _build_nc():
    import concourse.bass as bass
    import concourse.tile as tile
    from concourse import mybir
    from contextlib import ExitStack

    DT = mybir.dt.float32
    HT = mybir.dt.float16
    XDT = getattr(mybir.dt, X_DT)
    WDT = getattr(mybir.dt, W_DT)
    OHDT = getattr(mybir.dt, OH_DT)
    AF = mybir.ActivationFunctionType

    nc = bass.Bass()
    const_d = nc.declare_dram_parameter(
        "const", [1, PL * Q + B + 128], HT, isOutput=False)
    x_d = nc.declare_dram_parameter("xin", [2, 128, (KT // 2) * B], XDT,
                                    isOutput=False)
    w_d = nc.declare_dram_parameter("win", [128, PL * KT * Q], WDT,
                                    isOutput=False)
    oh_d = nc.declare_dram_parameter("ohin", [128, PL * C], OHDT,
                                     isOutput=False)
    out_d = nc.declare_dram_parameter("out", [B, PL, C], HT, isOutput=True)

    OFF_BIAS = 0
    OFF_ONES = PL * Q
    OFF_NEG = PL * Q + B

    with ExitStack() as ctx:
        tc = ctx.enter_context(tile.TileContext(nc))
        singles = ctx.enter_context(tc.tile_pool(name="singles", bufs=1))
        ps_lin = ctx.enter_context(
            tc.tile_pool(name="ps_lin", bufs=3, space=bass.MemorySpace.PSUM))
        ps_sum = ctx.enter_context(
            tc.tile_pool(name="ps_sum", bufs=1, space=bass.MemorySpace.PSUM))
        ps_out = ctx.enter_context(
            tc.tile_pool(name="ps_out", bufs=2, space=bass.MemorySpace.PSUM))

        def fresh(shape, dtype, tag):
            return singles.tile(shape, dtype, tag=tag, name=tag)

        # ---- input DMAs (SP queue, in tuned arrival order) ----------
        x_a = fresh([128, KT // 2, B], XDT, "x_a")
        x_b = fresh([128, KT // 2, B], XDT, "x_b")
        const_sb = fresh([1, PL * Q + B + 128], HT, "const")
        w_tiles = {}
        oh_tiles = {}

        def dma_w_group(gi):
            name, ps = W_GROUPS[gi]
            t = fresh([128, len(ps) * KT, Q], WDT, name)
            nc.sync.dma_start(
                out=t[:],
                in_=w_d[:, ps[0] * KT * Q:(ps[-1] + 1) * KT * Q])
            for j, p in enumerate(ps):
                w_tiles[p] = (t, j)

        def dma_oh_group(gi):
            name, ps = OH_GROUPS[gi]
            t = fresh([128, len(ps), C], OHDT, name)
            nc.sync.dma_start(
                out=t[:], in_=oh_d[:, ps[0] * C:(ps[-1] + 1) * C])
            for j, p in enumerate(ps):
                oh_tiles[p] = (t, j)

        # queue pre-warm: a throwaway read absorbs the DMA engines'
        # cold-start stagger so x_a/w0 stream at full rate
        qwarm = fresh([128, (KT // 2) * B], XDT, "qwarm")
        nc.sync.dma_start(out=qwarm[:], in_=x_d[0])
        nc.sync.dma_start(out=const_sb[:], in_=const_d[:])
        nc.sync.dma_start(out=x_a[:], in_=x_d[0])
        dma_w_group(0)                     # w0
        nc.sync.dma_start(out=x_b[:], in_=x_d[1])
        dma_w_group(1)                     # w1-2
        dma_oh_group(0)                    # oh0-1
        dma_w_group(2)                     # w3-5
        dma_oh_group(1)                    # oh2-4
        dma_w_group(3)                     # w6-7
        dma_oh_group(2)                    # oh5-7

        def w_pair_slice(p, t2):
            t, j = w_tiles[p]
            return t[:, j * KT + 2 * t2:j * KT + 2 * t2 + 2, :]

        def x_pair_slice(t2):
            t = x_a if t2 < KT // 4 else x_b
            tt = t2 % (KT // 4)
            return t[:, 2 * tt:2 * tt + 2, :]

        def oh_slice(p, c0, cw):
            t, j = oh_tiles[p]
            return t[:, j, c0:c0 + cw]

        # ones column for the sumexp matmuls (ACT-made, dep on x_a DMA)
        ones_col = fresh([128, 1], HT, "ones_col")
        nc.scalar.activation(out=ones_col[:], in_=x_a[:, 0, 0:1],
                             func=AF.Copy, bias=1.0, scale=0.0)

        # ---- per-partitioning pipeline ------------------------------
        og_tiles = {}
        # drain-engine per og tile (pair, bt): DVE ~11 drains, ACT ~5,
        # interleaved in time so neither engine falls behind
        ACT_TILES = {(1, 0), (1, 1), (3, 0), (3, 1)}
        for p in range(PL):
            pair = p // 2

            psum_lin = ps_lin.tile([128, B], DT)
            nc.tensor.matmul(
                psum_lin[:],
                const_sb[:, OFF_BIAS + p * Q:OFF_BIAS + (p + 1) * Q],
                const_sb[:, OFF_ONES:OFF_ONES + B],
                start=True, stop=False)
            for t2 in range(KT // 2):
                # DoubleRow: two 128-deep k-tiles contract per matmul
                nc.tensor.matmul(
                    psum_lin[:], w_pair_slice(p, t2), x_pair_slice(t2),
                    start=False, stop=(t2 == KT // 2 - 1),
                    perf_mode=mybir.MatmulPerfMode.DoubleRow)


            expT = fresh([128, B], HT, f"exp{p}")
            nc.scalar.activation(out=expT[:], in_=psum_lin[:], func=AF.Exp)

            psum_sum = ps_sum.tile([1, B], DT)
            nc.tensor.matmul(
                psum_sum[:], ones_col[:], expT[:],
                start=True, stop=True)
            lse = fresh([1, B], HT, f"lse{p}")
            nc.scalar.activation(out=lse[:], in_=psum_sum[:], func=AF.Ln)

            # -lse folded into the linear psum: psum[q,b] += (-1)*lse[b]
            nc.tensor.matmul(
                psum_lin[:],
                const_sb[:, OFF_NEG:OFF_NEG + 128],
                lse[:],
                start=False, stop=True, skip_group_check=True)

            linT = fresh([128, B], HT, f"lin{p}")
            nc.vector.tensor_copy(linT[:], psum_lin[:])

            for bt in range(BT):
                bsl = slice(bt * 128, (bt + 1) * 128)
                if p % 2 == 0:
                    og_tiles[(pair, bt)] = fresh([128, 2, C], HT,
                                                 f"og{pair}_{bt}")
                og = og_tiles[(pair, bt)]
                psum_out = ps_out.tile([128, 1024], DT)
                for (c0, cw) in C_CHUNKS:
                    nc.tensor.matmul(
                        psum_out[:, c0:c0 + cw],
                        linT[:, bsl],
                        oh_slice(p, c0, cw),
                        start=True, stop=True,
                        perf_mode=(mybir.MatmulPerfMode.DoublePixel
                                   if GATHER_DP else None))

                if (pair, bt) not in ACT_TILES:
                    nc.vector.tensor_copy(og[:, p % 2, :], psum_out[:, :C])
                else:
                    nc.scalar.activation(out=og[:, p % 2, :],
                                         in_=psum_out[:, :C], func=AF.Copy)
                if p % 2 == 1:
                    # out-DMAs dispatched from the otherwise-idle GpSimd
                    # sequencer (own HWDGE queue, overlaps the input queue)
                    nc.gpsimd.dma_start(
                        out=out_d[bsl, p - 1:p + 1, :],
                        in_=og[:])

    _install_wait_split(nc)
    return nc


def _install_wait_split(nc):
    """Walrus fits ONE sync-wait per instruction.  For every instruction
    carrying N>1 waits, keep the last and emit N-1 standalone
    EventSemaphore instructions (same engine, one wait each) before it.
    Engines execute their stream in order, so this is semantically
    identical.  Applied at serialization time so every consumer of
    nc.to_json_bytes() sees the legal form."""
    import json

    orig = nc.to_json_bytes

    def patched():
        m = json.loads(orig())
        n_split = 0
        for fn in m["functions"]:
            for bb in fn["blocks"]:
                out = []
                for inst in bb["instructions"]:
                    si = inst.get("sync_info")
                    if si and si.get("on_wait") and len(si["on_wait"]) > 1:
                        waits = si["on_wait"]
                        head, keep = waits[:-1], waits[-1:]
                        for j, w in enumerate(head):
                            out.append({
                                "debug": inst.get("debug", 0),
                                "engine": inst["engine"],
                                "ins": [],
                                "name": f"{inst['name']}-ws{j}",
                                "opcode": "EventSemaphore",
                                "outs": [],
                                "sync_info": {
                                    "on_update": [],
                                    "on_wait": [w],
                                },
                            })
                            n_split += 1
                        si["on_wait"] = keep
                    out.append(inst)
                bb["instructions"] = out
        return json.dumps(m).encode()

    nc.to_json_bytes = patched


def _host_inputs(x, W, b, part_idx):
    """Build the 8 per-core input maps."""
    x_np = _np_dt(X_DT)
    w_np = _np_dt(W_DT)
    oh_np = _np_dt(OH_DT)

    # x: [2, 128 f_in, (KT/2)*B]
    xh = np.ascontiguousarray(
        x.T.reshape(2, KT // 2, 128, B).transpose(0, 2, 1, 3)
         .reshape(2, 128, (KT // 2) * B)).astype(x_np)
    qs = np.arange(Q)
    in_maps = []
    for i in range(NCORES):
        sl = slice(i * PL, (i + 1) * PL)
        Wc = W[sl]                                     # [PL, Q, F]
        # -> [128 f_in, PL, KT, Q] -> [128, PL*KT*Q]
        wh = np.ascontiguousarray(
            Wc.transpose(2, 0, 1).reshape(KT, 128, PL, Q)
              .transpose(1, 2, 0, 3).reshape(128, PL * KT * Q)).astype(w_np)
        idx = part_idx[sl]                             # [PL, C]
        # oh[q, p, c] -> [128, PL*C]
        oh = np.ascontiguousarray(
            (idx[None, :, :] == qs[:, None, None])
            .reshape(128, PL * C)).astype(oh_np)
        const = np.zeros((1, PL * Q + B + 128), dtype=np.float16)
        const[0, :PL * Q] = b[sl].reshape(-1).astype(np.float16)
        const[0, PL * Q:PL * Q + B] = 1.0
        const[0, PL * Q + B:] = -1.0
        in_maps.append({
            "const": const,
            "xin": xh,
            "win": wh,
            "ohin": oh,
        })
    return in_maps


def kernel(x, W, b, part_idx, _trace=False):
    from concourse.bass_utils import run_bass_kernel_spmd

    x = np.asarray(x, dtype=np.float32)
    W = np.asarray(W, dtype=np.float32)
    b = np.asarray(b, dtype=np.float32)
    part_idx = np.asarray(part_idx)

    nc = _build_nc()
    in_maps = _host_inputs(x, W, b, part_idx)
    res = run_bass_kernel_spmd(nc, in_maps, list(range(NCORES)),
                               trace=_trace)
    out = np.concatenate(
        [r["out"].astype(np.float32) for r in res.results], axis=1)
    if _trace:
        return out, res
    return out


# revision 61
# speedup vs baseline: 1.0374x; 1.0097x over previous
"""Trainium2 Bass kernel for nn_CombinatorialClassifier.

Computation (reference):
    logits = einsum('bf,pqf->bpq', x, W) + b        # [B,P,Q]
    logp   = log_softmax(logits, axis=2)            # [B,P,Q]
    out    = take_along_axis(logp, part_idx, 2)     # [B,P,C]

Shapes: B=256, P=64, Q=128, C=1000, F=2048.

Sharding: expert-parallel over P across 8 cores (8 partitionings per
core).  Each core reads the full x and its W/b/part_idx slice and
writes its disjoint [B, 8, C] slice of the output.  No collectives.

Per-core dataflow (PSUM orientation [q, b] for the linear part):
  - W arrives per-p (one DMA per partitioning, x combined with W0) so
    p=0's matmuls start after ~1.7MB of DMA instead of the full 7MB.
  - lin group: bias K=1 matmul opens, 16 k-tile matmuls accumulate;
    after exp/sumexp/ln a final K=1 matmul (negones[q] x lse[b]) adds
    -lse[b] to every element, so psum_lin holds log-softmax directly.
  - gather: psum_out[b, c] = logpT[q,b].T @ OH[q,c] with the one-hot
    OH built on the HOST (exact 0/1) and shipped as an input; the
    PSUM->SBUF drains are then PLAIN dtype-cast copies (single dep),
    alternating DVE / ACT per p-pair.
  - output staged in SBUF fp16 (halves out traffic); host casts back
    to fp32.

This walrus build fits only ONE sync-wait per instruction.  Instead of
contorting the dataflow, _install_wait_split post-processes the
serialized IR: every instruction with N>1 waits keeps one and gets
N-1 standalone single-wait EventSemaphore instructions immediately
before it on the same engine/queue — semantically identical.
"""

import numpy as np

B, P, Q, C, F = 256, 64, 128, 1000, 2048
NCORES = 8
PL = P // NCORES          # partitionings per core
KT = F // 128             # contraction tiles
BT = B // 128             # batch tiles for the gather matmul
C_CHUNKS = [(0, 512), (512, C - 512)]
KW = B + Q                # combined x|w0 column block per k-tile

# dtypes (mybir names) for the big streamed operands
X_DT = "float8e4"         # x (moving in the main matmul; DoubleRow)
W_DT = "float8e4"         # W (stationary in main matmul; DoubleRow)
OH_DT = "float8e4"        # one-hot gather matrix (moving; 0/1 exact)
GATHER_DP = True          # DoublePixel perf mode on the gather matmuls

# input DMA grouping: (name, list of p's) for W and OH group tiles,
# interleaved so each operand lands just before the PE needs it
W_GROUPS = [("w0", [0]), ("wa", [1, 2]), ("wb", [3, 4, 5]), ("wc", [6, 7])]
OH_GROUPS = [("oha", [0, 1]), ("ohb", [2, 3, 4]), ("ohc", [5, 6, 7])]


def _np_dt(name):
    import ml_dtypes
    return {
        "float16": np.float16,
        "bfloat16": ml_dtypes.bfloat16,
        "float8e4": ml_dtypes.float8_e4m3fn,
        "float32": np.float32,
    }[name]


def _build_nc():
    import concourse.bass as bass
    import concourse.tile as tile
    from concourse import mybir
    from contextlib import ExitStack

    DT = mybir.dt.float32
    HT = mybir.dt.float16
    XDT = getattr(mybir.dt, X_DT)
    WDT = getattr(mybir.dt, W_DT)
    OHDT = getattr(mybir.dt, OH_DT)
    AF = mybir.ActivationFunctionType

    nc = bass.Bass()
    const_d = nc.declare_dram_parameter(
        "const", [1, PL * Q + B + 128], HT, isOutput=False)
    x_d = nc.declare_dram_parameter("xin", [2, 128, (KT // 2) * B], XDT,
                                    isOutput=False)
    w_d = nc.declare_dram_parameter("win", [128, PL * KT * Q], WDT,
                                    isOutput=False)
    oh_d = nc.declare_dram_parameter("ohin", [128, PL * C], OHDT,
                                     isOutput=False)
    out_d = nc.declare_dram_parameter("out", [B, PL, C], HT, isOutput=True)

    OFF_BIAS = 0
    OFF_ONES = PL * Q
    OFF_NEG = PL * Q + B

    with ExitStack() as ctx:
        tc = ctx.enter_context(tile.TileContext(nc))
        singles = ctx.enter_context(tc.tile_pool(name="singles", bufs=1))
        ps_lin = ctx.enter_context(
            tc.tile_pool(name="ps_lin", bufs=3, space=bass.MemorySpace.PSUM))
        ps_sum = ctx.enter_context(
            tc.tile_pool(name="ps_sum", bufs=1, space=bass.MemorySpace.PSUM))
        ps_out = ctx.enter_context(
            tc.tile_pool(name="ps_out", bufs=2, space=bass.MemorySpace.PSUM))

        def fresh(shape, dtype, tag):
            return singles.tile(shape, dtype, tag=tag, name=tag)

        # ---- input DMAs (SP queue, in tuned arrival order) ----------
        x_a = fresh([128, KT // 2, B], XDT, "x_a")
        x_b = fresh([128, KT // 2, B], XDT, "x_b")
        const_sb = fresh([1, PL * Q + B + 128], HT, "const")
        w_tiles = {}
        oh_tiles = {}

        def dma_w_group(gi):
            name, ps = W_GROUPS[gi]
            t = fresh([128, len(ps) * KT, Q], WDT, name)
            nc.sync.dma_start(
                out=t[:],
                in_=w_d[:, ps[0] * KT * Q:(ps[-1] + 1) * KT * Q])
            for j, p in enumerate(ps):
                w_tiles[p] = (t, j)

        def dma_oh_group(gi):
            name, ps = OH_GROUPS[gi]
            t = fresh([128, len(ps), C], OHDT, name)
            nc.sync.dma_start(
                out=t[:], in_=oh_d[:, ps[0] * C:(ps[-1] + 1) * C])
            for j, p in enumerate(ps):
                oh_tiles[p] = (t, j)

        nc.sync.dma_start(out=const_sb[:], in_=const_d[:])
        nc.sync.dma_start(out=x_a[:], in_=x_d[0])
        dma_w_group(0)                     # w0
        nc.sync.dma_start(out=x_b[:], in_=x_d[1])
        dma_w_group(1)                     # w1-2
        dma_oh_group(0)                    # oh0-1
        dma_w_group(2)                     # w3-5
        dma_oh_group(1)                    # oh2-4
        dma_w_group(3)                     # w6-7
        dma_oh_group(2)                    # oh5-7

        def w_pair_slice(p, t2):
            t, j = w_tiles[p]
            return t[:, j * KT + 2 * t2:j * KT + 2 * t2 + 2, :]

        def x_pair_slice(t2):
            t = x_a if t2 < KT // 4 else x_b
            tt = t2 % (KT // 4)
            return t[:, 2 * tt:2 * tt + 2, :]

        def oh_slice(p, c0, cw):
            t, j = oh_tiles[p]
            return t[:, j, c0:c0 + cw]

        # ones column for the sumexp matmuls (ACT-made, dep on x_a DMA)
        ones_col = fresh([128, 1], HT, "ones_col")
        nc.scalar.activation(out=ones_col[:], in_=x_a[:, 0, 0:1],
                             func=AF.Copy, bias=1.0, scale=0.0)

        # ---- per-partitioning pipeline ------------------------------
        og_tiles = {}
        # drain-engine per og tile (pair, bt): DVE ~11 drains, ACT ~5,
        # interleaved in time so neither engine falls behind
        ACT_TILES = {(1, 0), (1, 1), (3, 0), (3, 1)}
        for p in range(PL):
            pair = p // 2

            psum_lin = ps_lin.tile([128, B], DT)
            nc.tensor.matmul(
                psum_lin[:],
                const_sb[:, OFF_BIAS + p * Q:OFF_BIAS + (p + 1) * Q],
                const_sb[:, OFF_ONES:OFF_ONES + B],
                start=True, stop=False)
            for t2 in range(KT // 2):
                # DoubleRow: two 128-deep k-tiles contract per matmul
                nc.tensor.matmul(
                    psum_lin[:], w_pair_slice(p, t2), x_pair_slice(t2),
                    start=False, stop=(t2 == KT // 2 - 1),
                    perf_mode=mybir.MatmulPerfMode.DoubleRow)


            expT = fresh([128, B], HT, f"exp{p}")
            nc.scalar.activation(out=expT[:], in_=psum_lin[:], func=AF.Exp)

            psum_sum = ps_sum.tile([1, B], DT)
            nc.tensor.matmul(
                psum_sum[:], ones_col[:], expT[:],
                start=True, stop=True)
            lse = fresh([1, B], HT, f"lse{p}")
            nc.scalar.activation(out=lse[:], in_=psum_sum[:], func=AF.Ln)

            # -lse folded into the linear psum: psum[q,b] += (-1)*lse[b]
            nc.tensor.matmul(
                psum_lin[:],
                const_sb[:, OFF_NEG:OFF_NEG + 128],
                lse[:],
                start=False, stop=True, skip_group_check=True)

            linT = fresh([128, B], HT, f"lin{p}")
            nc.vector.tensor_copy(linT[:], psum_lin[:])

            for bt in range(BT):
                bsl = slice(bt * 128, (bt + 1) * 128)
                if p % 2 == 0:
                    og_tiles[(pair, bt)] = fresh([128, 2, C], HT,
                                                 f"og{pair}_{bt}")
                og = og_tiles[(pair, bt)]
                psum_out = ps_out.tile([128, 1024], DT)
                for (c0, cw) in C_CHUNKS:
                    nc.tensor.matmul(
                        psum_out[:, c0:c0 + cw],
                        linT[:, bsl],
                        oh_slice(p, c0, cw),
                        start=True, stop=True,
                        perf_mode=(mybir.MatmulPerfMode.DoublePixel
                                   if GATHER_DP else None))

                if (pair, bt) not in ACT_TILES:
                    nc.vector.tensor_copy(og[:, p % 2, :], psum_out[:, :C])
                else:
                    nc.scalar.activation(out=og[:, p % 2, :],
                                         in_=psum_out[:, :C], func=AF.Copy)
                if p % 2 == 1:
                    # out-DMAs dispatched from the otherwise-idle GpSimd
                    # sequencer (own HWDGE queue, overlaps the input queue)
                    nc.gpsimd.dma_start(
                        out=out_d[bsl, p - 1:p + 1, :],
                        in_=og[:])

    _install_wait_split(nc)
    return nc


def _install_wait_split(nc):
    """Walrus fits ONE sync-wait per instruction.  For every instruction
    carrying N>1 waits, keep the last and emit N-1 standalone
    EventSemaphore instructions (same engine, one wait each) before it.
    Engines execute their stream in order, so this is semantically
    identical.  Applied at serialization time so every consumer of
    nc.to_json_bytes() sees the legal form."""
    import json

    orig = nc.to_json_bytes

    def patched():
        m = json.loads(orig())
        n_split = 0
        for fn in m["functions"]:
            for bb in fn["blocks"]:
                out = []
                for inst in bb["instructions"]:
                    si = inst.get("sync_info")
                    if si and si.get("on_wait") and len(si["on_wait"]) > 1:
                        waits = si["on_wait"]
                        head, keep = waits[:-1], waits[-1:]
                        for j, w in enumerate(head):
                            out.append({
                                "debug": inst.get("debug", 0),
                                "engine": inst["engine"],
                                "ins": [],
                                "name": f"{inst['name']}-ws{j}",
                                "opcode": "EventSemaphore",
                                "outs": [],
                                "sync_info": {
                                    "on_update": [],
                                    "on_wait": [w],
                                },
                            })
                            n_split += 1
                        si["on_wait"] = keep
                    out.append(inst)
                bb["instructions"] = out
        return json.dumps(m).encode()

    nc.to_json_bytes = patched


def _host_inputs(x, W, b, part_idx):
    """Build the 8 per-core input maps."""
    x_np = _np_dt(X_DT)
    w_np = _np_dt(W_DT)
    oh_np = _np_dt(OH_DT)

    # x: [2, 128 f_in, (KT/2)*B]
    xh = np.ascontiguousarray(
        x.T.reshape(2, KT // 2, 128, B).transpose(0, 2, 1, 3)
         .reshape(2, 128, (KT // 2) * B)).astype(x_np)
    qs = np.arange(Q)
    in_maps = []
    for i in range(NCORES):
        sl = slice(i * PL, (i + 1) * PL)
        Wc = W[sl]                                     # [PL, Q, F]
        # -> [128 f_in, PL, KT, Q] -> [128, PL*KT*Q]
        wh = np.ascontiguousarray(
            Wc.transpose(2, 0, 1).reshape(KT, 128, PL, Q)
              .transpose(1, 2, 0, 3).reshape(128, PL * KT * Q)).astype(w_np)
        idx = part_idx[sl]                             # [PL, C]
        # oh[q, p, c] -> [128, PL*C]
        oh = np.ascontiguousarray(
            (idx[None, :, :] == qs[:, None, None])
            .reshape(128, PL * C)).astype(oh_np)
        const = np.zeros((1, PL * Q + B + 128), dtype=np.float16)
        const[0, :PL * Q] = b[sl].reshape(-1).astype(np.float16)
        const[0, PL * Q:PL * Q + B] = 1.0
        const[0, PL * Q + B:] = -1.0
        in_maps.append({
            "const": const,
            "xin": xh,
            "win": wh,
            "ohin": oh,
        })
    return in_maps


def kernel(x, W, b, part_idx, _trace=False):
    from concourse.bass_utils import run_bass_kernel_spmd

    x = np.asarray(x, dtype=np.float32)
    W = np.asarray(W, dtype=np.float32)
    b = np.asarray(b, dtype=np.float32)
    part_idx = np.asarray(part_idx)

    nc = _build_nc()
    in_maps = _host_inputs(x, W, b, part_idx)
    res = run_bass_kernel_spmd(nc, in_maps, list(range(NCORES)),
                               trace=_trace)
    out = np.concatenate(
        [r["out"].astype(np.float32) for r in res.results], axis=1)
    if _trace:
        return out, res
    return out
